# revision 9
# baseline (speedup 1.0000x reference)
"""Trainium2 Bass kernel for GNN message passing (8 NeuronCores, SPMD).

    out = segment_sum(x[src] @ W, tgt, N) + x @ W_self

Key algebraic identity: segment_sum(x[src] @ W, tgt) = segment_sum(x[src], tgt) @ W,
so the per-edge matmul hoists out of the reduction (21 GFLOP -> 6.6 GFLOP).

Sharding: target nodes are split into 8 contiguous ranges of 12500 (one per
core); edges are bucketed to the core owning their target. x is replicated in
every core's HBM so each core gathers arbitrary source rows locally (no
cross-core halo exchange needed under the full-I/O contract).

Per core, working transposed throughout (out.T = W.T @ hT + W_self.T @ xT):
  - targets are processed in 98 windows of 128 nodes
  - per 128-edge tile: G[e,f] = x[src_e] via indirect DMA gather,
    S[e,n] = onehot(tgt_local) built by a DVE is_equal against an iota,
    hT (PSUM) += matmul(lhsT=G, rhs=S)   # = sum_e G[e,f] S[e,n]
  - outT (PSUM) = matmul(lhsT=W, rhs=hT) + matmul(lhsT=W_self, rhs=xT_window)
The host transposes per-core [128, 12544] outputs back and concatenates.

v2 (this file) vs the fp32 baseline:
  - SWDGE instruction merging: the baseline issued 784 dma_gathers (one per
    window x chunk) over 2 queues; SWDGE descriptor prep is ~994ns fixed +
    0.34ns/desc per instruction, so ~392 x 1.08us = ~424us serialized on one
    queue -- the real baseline bottleneck (measured 430us). Now one gather
    covers 7 windows' worth of one chunk: 56 calls over 4 queues.
  - bf16 data path: x gathered as bf16 (256B rows -> half the HBM traffic),
    one-hot S in bf16 (DVE 2x eligible), aggregation as single bf16 matmuls
    (1 cy/row vs fp32's 4), weights applied in bf16 at N=512. PSUM
    accumulation stays fp32. rel err ~4e-3 (well under the 2e-2 gate).
"""

import numpy as np

P = 128
D = 128
N_NODES = 100000
N_CORES = 8
N_LOC = N_NODES // N_CORES          # 12500
N_WIN = (N_LOC + P - 1) // P        # 98
N_PAD = N_WIN * P                   # 12544
G_WIN = 7                           # windows per merged gather call (98 = 14*7)

# dma_gather uses int16 row indices, so x is addressed through 4 overlapping
# 32768-row chunks; every source row is reachable from >=1 chunk and rows in
# overlap regions can go to either side, which lets the host balance the four
# per-window runs under the per-chunk tile cap.
N_CHUNK = 4
CHUNK_SPAN = 32768
CHUNK_BASE = [0, 22411, 44822, N_NODES - CHUNK_SPAN]

_program_cache: dict = {}


def _build_program(
    t_win: int,
    reps: int = 1,
    n_queues: int = 4,
    g_win: int = G_WIN,
    w_group: int = 4,
    max_call_idx: int = 1024,
):
    import concourse.bass as bass
    import concourse.mybir as mybir
    import concourse.tile as tile
    from concourse.bacc import Bacc

    f32 = mybir.dt.float32
    bf16 = mybir.dt.bfloat16
    t_c = t_win // N_CHUNK
    assert t_c * N_CHUNK == t_win
    assert N_WIN % g_win == 0
    n_grp = N_WIN // g_win
    cap_call = g_win * t_c * P          # indices per merged gather call
    cols16 = cap_call // 16             # int16 idx columns per call

    # consts packed as one tensor/DMA so consumers wait on a single semaphore:
    # [idx16 (int16 bits) | tl (bf16) | iota (bf16) | W (bf16) | W_self (bf16)]
    t_tot = N_WIN * t_win
    idx_cols = n_grp * N_CHUNK * cols16 // 2   # as int32 columns
    tl_cols = t_tot // 2
    iota_cols = t_win * P // 2
    w_cols = P // 2
    k_const = idx_cols + tl_cols + iota_cols + 2 * w_cols

    # Bacc (not raw Bass): its finalize() legalizes sync waits — TRN2 allows
    # at most one semaphore wait per instruction and walrus rejects more.
    nc = Bacc(num_swdge_queues=n_queues)
    x_d = nc.declare_dram_parameter("xb", [N_NODES, D], bf16, isOutput=False)
    xT_d = nc.declare_dram_parameter("xTb", [D, N_PAD], bf16, isOutput=False)
    consts_d = nc.declare_dram_parameter(
        "consts", [P, k_const], mybir.dt.int32, isOutput=False
    )
    outT_d = nc.declare_dram_parameter("outT", [D, N_PAD], bf16, isOutput=True)

    with tile.TileContext(nc) as tc:
        with (
            tc.tile_pool(name="const", bufs=1) as cpool,
            tc.tile_pool(name="gath", bufs=3) as gpool,
            tc.tile_pool(name="spool", bufs=3) as spool,
            tc.tile_pool(name="wtile", bufs=3) as wpool,
            tc.tile_pool(name="psum", bufs=2, space="PSUM") as psum,
            tc.tile_pool(name="opsum", bufs=2, space="PSUM") as opsum,
            tc.tile_pool(name="scratch", bufs=1, space="PSUM") as scratch_pool,
        ):
            scratch_ps = scratch_pool.tile([1, 1], f32)
            const_sb = cpool.tile([P, k_const], mybir.dt.int32)
            nc.sync.dma_start(const_sb[:], consts_d[:])
            a0 = 0
            idx16_sb = const_sb[:, a0 : a0 + idx_cols].bitcast(mybir.dt.int16)
            a0 += idx_cols
            tl_sb = const_sb[:, a0 : a0 + tl_cols].bitcast(bf16)
            a0 += tl_cols
            iota_sb = const_sb[:, a0 : a0 + iota_cols].bitcast(bf16)
            a0 += iota_cols
            w_sb = const_sb[:, a0 : a0 + w_cols].bitcast(bf16)
            a0 += w_cols
            ws_sb = const_sb[:, a0 : a0 + w_cols].bitcast(bf16)
            # dense step-1 [128, t_win, 128] view of the tiled iota pattern
            iota3 = iota_sb.rearrange("p (t n) -> p t n", t=t_win)

            for rep in range(reps):
                for grp in range(n_grp):
                    G_grp = gpool.tile([P, N_CHUNK, g_win * t_c, D], bf16)
                    for c in range(N_CHUNK):
                        # gather via dma_gather (int16 idx against a 32768-row
                        # chunk of x): slot (p, s) = row idx[s*128+p] of the
                        # chunk. One call covers up to max_call_idx indices
                        # (HW faults above ~1024/call) spanning several
                        # windows' chunk-c runs — SWDGE prep is ~1us fixed per
                        # call, so merging calls keeps descriptor generation
                        # off the critical path.
                        call = (grp * N_CHUNK + c) * cols16
                        m_per = max(1, max_call_idx // (t_c * P))
                        for m0 in range(0, g_win, m_per):
                            m1 = min(m0 + m_per, g_win)
                            n_idx = (m1 - m0) * t_c * P
                            nc.gpsimd.dma_gather(
                                G_grp[:, c, m0 * t_c : m1 * t_c, :],
                                x_d[CHUNK_BASE[c] : CHUNK_BASE[c] + CHUNK_SPAN, :],
                                idx16_sb[
                                    :,
                                    call + m0 * t_c * 8 : call + m1 * t_c * 8,
                                ],
                                n_idx,
                                n_idx,
                                D,
                                queue_num=c % n_queues,
                            )
                    # one S build covers the whole group's windows — the DVE
                    # op overhead (~120 cy) amortizes over g_win*t_win*128
                    # columns. 1x mode regardless (the broadcast tl operand's
                    # stride-0 last dim disqualifies 2x packing).
                    S_grp = spool.tile([P, g_win * t_win, P], bf16)
                    nc.vector.tensor_tensor(
                        out=S_grp[:].rearrange(
                            "p (g t) n -> p g t n", g=g_win
                        ),
                        in0=iota3.unsqueeze(1).to_broadcast(
                            [P, g_win, t_win, P]
                        ),
                        in1=tl_sb[
                            :, grp * g_win * t_win : (grp + 1) * g_win * t_win
                        ]
                        .rearrange("p (g t) -> p g t", g=g_win)
                        .unsqueeze(3)
                        .to_broadcast([P, g_win, t_win, P]),
                        op=mybir.AluOpType.is_equal,
                    )
                    # matmuls can carry only ONE sync wait; the first real
                    # matmul below depends on both S_grp (DVE) and G_grp
                    # (DMA). This throwaway 1x1 matmul makes the PE queue
                    # observe the DVE tick first so each real matmul needs
                    # a single wait.
                    nc.tensor.matmul(
                        scratch_ps[:],
                        lhsT=S_grp[:, 0, 0:1],
                        rhs=S_grp[:, 0, 0:1],
                        start=True,
                        stop=True,
                    )
                    for m in range(g_win):
                        w = grp * g_win + m
                        hT_ps = psum.tile([D, P], f32)
                        for t in range(t_win):
                            c, j = divmod(t, t_c)
                            nc.tensor.matmul(
                                hT_ps[:],
                                lhsT=G_grp[:, c, m * t_c + j, :],
                                rhs=S_grp[:, m * t_win + t, :],
                                start=(t == 0),
                                stop=(t == t_win - 1),
                            )
                        # grouped W-apply: stage hT of w_group windows side by
                        # side, then stream both weight matmuls at N =
                        # w_group*128. ACT does the PSUM->SBUF copies (with
                        # f32->bf16 cast); DVE is saturated building S.
                        gi = w % w_group
                        if gi == 0:
                            n_in_grp = min(w_group, N_WIN - w)
                            hT_sb = wpool.tile([D, w_group * P], bf16, tag="hT")
                        nc.scalar.copy(hT_sb[:, gi * P : (gi + 1) * P], hT_ps[:])
                        if gi == n_in_grp - 1:
                            w0 = w - gi
                            span = n_in_grp * P
                            xT_sb = wpool.tile([D, w_group * P], bf16, tag="xT")
                            nc.sync.dma_start(
                                xT_sb[:, :span], xT_d[:, w0 * P : w0 * P + span]
                            )
                            outT_ps = opsum.tile([D, w_group * P], f32)
                            nc.tensor.matmul(
                                outT_ps[:, :span],
                                lhsT=w_sb,
                                rhs=hT_sb[:, :span],
                                start=True,
                                stop=False,
                            )
                            nc.tensor.matmul(
                                outT_ps[:, :span],
                                lhsT=ws_sb,
                                rhs=xT_sb[:, :span],
                                start=False,
                                stop=True,
                            )
                            o_sb = wpool.tile([D, w_group * P], bf16, tag="o")
                            nc.scalar.copy(o_sb[:, :span], outT_ps[:, :span])
                            nc.sync.dma_start(
                                outT_d[:, w0 * P : w0 * P + span], o_sb[:, :span]
                            )

    nc.finalize()
    return nc


def _prep_inputs(x, edge_index, W, W_self, g_win=G_WIN):
    """Host-side sharding: bucket+sort edges by target core/window, pad to a
    uniform tile count, build per-core input maps (bf16 data path)."""
    import ml_dtypes

    bf = ml_dtypes.bfloat16
    x = np.ascontiguousarray(np.asarray(x, dtype=np.float32))
    W = np.ascontiguousarray(np.asarray(W, dtype=np.float32))
    W_self = np.ascontiguousarray(np.asarray(W_self, dtype=np.float32))
    ei = np.asarray(edge_index)
    src = ei[0].astype(np.int64)
    tgt = ei[1].astype(np.int64)

    order = np.argsort(tgt, kind="stable")
    src_s = src[order].astype(np.int64)
    tgt_s = tgt[order]
    core = tgt_s // N_LOC
    wloc = (tgt_s - core * N_LOC) // P
    gw = (core * N_WIN + wloc).astype(np.int64)
    counts = np.bincount(gw, minlength=N_CORES * N_WIN)
    t_win_data = max(1, int(np.ceil(counts.max() / P)))
    t_c = max(2, (t_win_data + N_CHUNK - 1) // N_CHUNK)

    # chunk feasibility per edge: lo = highest chunk with base <= s,
    # hi = lowest chunk with s < base + CHUNK_SPAN (consecutive range)
    bases = np.asarray(CHUNK_BASE, np.int64)
    lo = np.searchsorted(bases, src_s, side="right") - 1
    hi = np.searchsorted(bases + CHUNK_SPAN, src_s, side="right")
    starts = np.concatenate([[0], np.cumsum(counts)])
    tl_val = (tgt_s - (core * N_LOC + wloc * P)).astype(np.float32)

    while True:
        cap = t_c * P
        t_win = N_CHUNK * t_c
        t_tot = N_WIN * t_win
        # linear per-(core*window, chunk) index runs, pad idx 0
        idx_runs = np.zeros((N_CORES * N_WIN, N_CHUNK, cap), np.int16)
        tl_flat = np.full(N_CORES * t_tot * P, -1.0, np.float32)
        ok = True
        for g in range(N_CORES * N_WIN):
            a, b = starts[g], starts[g + 1]
            if b - a > N_CHUNK * cap:
                ok = False
                break
            s_g, hi_g, lo_g, tl_g = src_s[a:b], hi[a:b], lo[a:b], tl_val[a:b]
            taken = np.zeros(b - a, bool)
            for c in range(N_CHUNK):
                cand = (~taken) & (hi_g <= c) & (c <= lo_g)
                must = cand & (lo_g == c)
                n_must = int(must.sum())
                if n_must > cap:
                    ok = False
                    break
                sel = must.nonzero()[0]
                flex = (cand & ~must).nonzero()[0][: cap - n_must]
                pick = np.concatenate([sel, flex])
                taken[pick] = True
                n = pick.size
                # ascending source order within the run: the SDMA descriptors
                # then walk HBM addresses monotonically (row-buffer locality)
                pick = pick[np.argsort(s_g[pick], kind="stable")]
                idx_runs[g, c, :n] = (s_g[pick] - bases[c]).astype(np.int16)
                # tl slots for this chunk run (pads stay -1)
                base_slot = g * (t_win * P) + c * cap
                tl_flat[base_slot : base_slot + n] = tl_g[pick]
            if not ok or not taken.all():
                ok = ok and bool(taken.all())
                if not ok:
                    break
        if ok:
            break
        t_c += 1

    n_grp = N_WIN // g_win
    cap_call = g_win * cap
    tl_dev = tl_flat.reshape(N_CORES, t_tot, P).transpose(0, 2, 1)
    # tiled iota pattern [128, t_win*128]: column t*128+n holds n
    iota_big = np.tile(np.arange(P, dtype=np.float32), (P, t_win))
    x_b = np.ascontiguousarray(x.astype(bf))
    W_b = W.astype(bf)
    Ws_b = W_self.astype(bf)
    in_maps = []
    for c in range(N_CORES):
        runs = idx_runs[c * N_WIN : (c + 1) * N_WIN]  # [N_WIN, N_CHUNK, cap]
        # call (grp, ch) = concat over m of window (grp*g_win+m)'s chunk run;
        # slot s of a call lands at SBUF [s % 16, s // 16], replicated x8 so
        # every GPSIMD Q7 core sees its stripe
        calls = (
            runs.reshape(n_grp, g_win, N_CHUNK, cap)
            .transpose(0, 2, 1, 3)
            .reshape(n_grp * N_CHUNK, cap_call)
        )
        wrapped = calls.reshape(-1, cap_call // 16, 16).transpose(1, 0, 2)
        sb16 = np.tile(
            np.ascontiguousarray(wrapped.transpose(2, 1, 0)).reshape(16, -1), (8, 1)
        )
        xT_c = np.zeros((D, N_PAD), bf)
        xT_c[:, :N_LOC] = x_b[c * N_LOC : (c + 1) * N_LOC].T
        consts = np.concatenate(
            [
                np.ascontiguousarray(sb16).view(np.int32),
                np.ascontiguousarray(tl_dev[c].astype(bf)).view(np.int32),
                iota_big.astype(bf).view(np.int32),
                np.ascontiguousarray(W_b).view(np.int32),
                np.ascontiguousarray(Ws_b).view(np.int32),
            ],
            axis=1,
        )
        in_maps.append({"xb": x_b, "xTb": xT_c, "consts": consts})
    return in_maps, t_win


def run(x, edge_index, W, W_self, trace=False, **trace_kwargs):
    """Returns (output [100000,128] float32, BassKernelResults)."""
    from concourse import bass_utils

    in_maps, t_win = _prep_inputs(x, edge_index, W, W_self)
    nc = _program_cache.get(t_win)
    if nc is None:
        nc = _build_program(t_win)
        _program_cache[t_win] = nc
    # A NeuronCore occasionally comes up wedged from a previous session
    # (NRT_EXEC_UNIT_UNRECOVERABLE); the failed attempt itself clears it, so
    # one retry recovers.
    try:
        res = bass_utils.run_bass_kernel_spmd(
            nc, in_maps, core_ids=list(range(N_CORES)), trace=trace, **trace_kwargs
        )
    except Exception:
        res = bass_utils.run_bass_kernel_spmd(
            nc, in_maps, core_ids=list(range(N_CORES)), trace=trace, **trace_kwargs
        )
    out = np.empty((N_NODES, D), np.float32)
    for c in range(N_CORES):
        out[c * N_LOC : (c + 1) * N_LOC] = (
            res.results[c]["outT"].astype(np.float32).T[:N_LOC]
        )
    return out, res


def kernel(x, edge_index, W, W_self):
    out, _ = run(x, edge_index, W, W_self, trace=False)
    return out
